# revision 4
# baseline (speedup 1.0000x reference)
"""Per-pixel dynamic 7x7 filtering (BaseTextureDiffusion._diffusion_step)
on 8 Trainium2 NeuronCores.

out[b,c,h,w] = sum_k weights[b,c,k,h,w] * pad_edge(latent)[b,c,h+i,w+j],
k = i*7+j.

Sharding: the 48 (b,c) planes are independent -> 6 planes per core.
Latent is replicate-padded on host (tiny) so the device kernel does no
edge handling.

Device layout per core: partition dim = image rows (2 blocks of 128),
free dim = (plane, col) -> 1536 elems per partition per op.  Inputs are
shipped fp16 (halves HBM traffic vs f32; rel err ~6e-4 << 2e-2).

v1 engine split (vs the all-DVE baseline at ~169 us):
  - DVE computes ONLY the 49 per-tap products (fp16 2x mode, ~860 ns per
    [128,1536] tensor_mul -> ~84 us total).
  - The 48 adds/pixel move to the idle TensorE: each product tile is
    accumulated into PSUM via identity-stationary matmuls
    (psum[:,s] += I.T @ prod[:,s], 3 x N=512 per tap, fp16 moving
    ~213 ns each -> ~63 us total, parallel to DVE).  PSUM accumulates in
    f32, which also improves precision over the fp16 partial sums the
    baseline used.
  - ScalarE (idle) evicts PSUM -> SBUF f32; DMA stores to HBM.
Expected bound: weight DMA traffic (38.5 MB fp16/core) + latent tiles.
"""

import numpy as np

B, C, H, W = 2, 24, 256, 256
R = 7
PAD = R // 2
NCORES = 8
PLANES = B * C  # 48
PPC = PLANES // NCORES  # 6 planes per core
HP = H + 2 * PAD  # 262
WP = W + 2 * PAD  # 262
FD = PPC * W  # 1536 free elems per partition per op
NBANK = 512  # fp32 elems per PSUM bank (matmul output limit)
DTYPE = "f16"

_cache = {}


def _split_multi_waits(nc, max_waits: int = 1):
    """walrus CoreV3 codegen in this container rejects instructions carrying
    more than one sync wait ('Too many sync wait commands').  Legalize the
    module by hoisting extra waits onto same-engine NoOps inserted directly
    before the instruction (engine stalls at the nop first — semantics
    preserved, the instruction still executes only after all conditions)."""
    import concourse.mybir as mybir

    cnt = 0
    for f in nc.m.functions:
        for b in f.blocks:
            changed = False
            new_insts = []
            for inst in b.instructions:
                si = inst.sync_info
                if si is not None and len(si.on_wait) > max_waits:
                    waits = list(si.on_wait)
                    upds = list(si.on_update)
                    chunks = [
                        waits[i : i + max_waits]
                        for i in range(0, len(waits), max_waits)
                    ]
                    for chunk in chunks[:-1]:
                        nop = mybir.InstNoOp(
                            name=f"ws_nop_{cnt}", ins=[], outs=[]
                        )
                        cnt += 1
                        nop.engine = inst.engine
                        nop.sync_info = mybir.SyncInfo(
                            on_wait=chunk, on_update=[]
                        )
                        new_insts.append(nop)
                    inst.sync_info = mybir.SyncInfo(
                        on_wait=chunks[-1], on_update=upds
                    )
                    changed = True
                new_insts.append(inst)
            if changed:
                b.instructions = new_insts
    return nc


def build_nc(
    reps: int = 1,
    loop_reps: int | None = None,
):
    """Build the per-core Bass program (SPMD; all cores run the same NEFF).

    loop_reps: if set, wrap ONE rep body in a hardware For_i loop with this
    trip count (constant NEFF size for any count; used for timing).
    """
    import concourse.bass as bass
    import concourse.mybir as mybir
    from concourse.tile import TileContext

    dt = mybir.dt.float16
    dto = mybir.dt.float32

    nc = bass.Bass("TRN2", target_bir_lowering=False, debug=False, num_devices=NCORES)
    # Weights are pre-transposed on host to [row, k, plane, col] so each
    # (row-block, tap) DMA is contiguous per partition (2-dim AP).
    wt_r = nc.dram_tensor("wt", [H, R * R, PPC, W], dt, kind="ExternalInput").ap()
    lp = nc.dram_tensor("lp", [PPC, HP, WP], dt, kind="ExternalInput").ap()
    ident = nc.dram_tensor("ident", [128, 128], dt, kind="ExternalInput").ap()
    out = nc.dram_tensor("out", [PPC, H, W], dto, kind="ExternalOutput").ap()

    # Rows on the partition dim.
    lp_r = lp.rearrange("pl r d -> r pl d")  # [262, 6, 262]
    out_r = out.rearrange("pl r c -> r pl c")  # [256, 6, 256]

    with TileContext(nc) as tc:
        with tc.tile_pool(name="pool", bufs=1) as pool, tc.psum_pool(
            name="pspool", bufs=1
        ) as pspool:
            # Identity stationary for the PE accumulation matmuls; loaded
            # once, outside the timing loop.
            id_t = pool.tile([128, 128], dt, name="id_t", tag="id", bufs=1)
            nc.sync.dma_start(out=id_t[:], in_=ident)

            def rep_body(rep):
                for blk in range(H // 128):
                    r0 = blk * 128
                    # Row-shifted padded-latent tiles, loaded lazily right
                    # before the first tap that needs them.  DVE 2x mode
                    # needs 4B-aligned slices, so odd col shifts read a
                    # copy pre-shifted by one element (rso).
                    rs = {}
                    rso = {}

                    def need_row(i):
                        if i in rs:
                            return
                        t = pool.tile(
                            [128, PPC, WP], dt,
                            name=f"rs_{rep}_{blk}_{i}", tag=f"rs{i}", bufs=2,
                        )
                        nc.sync.dma_start(out=t[:], in_=lp_r[r0 + i : r0 + i + 128])
                        rs[i] = t
                        to = pool.tile(
                            [128, PPC, WP], dt,
                            name=f"rso_{rep}_{blk}_{i}", tag=f"rso{i}", bufs=2,
                        )
                        nc.sync.dma_start(
                            out=to[:, :, 0 : WP - 1],
                            in_=lp_r[r0 + i : r0 + i + 128, :, 1:WP],
                        )
                        rso[i] = to

                    psum_t = pspool.tile(
                        [128, FD], dto, name=f"ps_{rep}_{blk}", tag="ps", bufs=2,
                    )
                    for k in range(R * R):
                        i, j = divmod(k, R)
                        need_row(i)
                        wg = pool.tile(
                            [128, PPC, W], dt,
                            name=f"wg_{rep}_{blk}_{k}", tag="wg", bufs=8,
                        )
                        nc.sync.dma_start(out=wg[:], in_=wt_r[r0 : r0 + 128, k])
                        if j % 2 == 1:
                            x = rso[i][:, :, j - 1 : j - 1 + W]
                        else:
                            x = rs[i][:, :, j : j + W]
                        prod = pool.tile(
                            [128, FD], dt,
                            name=f"prod_{rep}_{blk}_{k}", tag="prod", bufs=4,
                        )
                        nc.vector.tensor_mul(
                            prod[:].rearrange("p (a b) -> p a b", a=PPC),
                            wg[:], x,
                        )
                        for s in range(FD // NBANK):
                            nc.tensor.matmul(
                                psum_t[:, s * NBANK : (s + 1) * NBANK],
                                id_t[:],
                                prod[:, s * NBANK : (s + 1) * NBANK],
                                start=(k == 0),
                                stop=(k == R * R - 1),
                            )
                    oacc = pool.tile(
                        [128, FD], dto, name=f"oacc_{rep}_{blk}", tag="oacc",
                        bufs=2,
                    )
                    nc.scalar.copy(out=oacc[:], in_=psum_t[:])
                    nc.sync.dma_start(
                        out=out_r[r0 : r0 + 128],
                        in_=oacc[:].rearrange("p (a b) -> p a b", a=PPC),
                    )

            if loop_reps is not None:
                with tc.For_i(0, loop_reps, 1):
                    rep_body(0)
            else:
                for rep in range(reps):
                    rep_body(rep)
    _split_multi_waits(nc)
    return nc


def _prep_inputs(latent, weights, dtype: str = DTYPE):
    npdt = np.float16
    lat = np.asarray(latent, dtype=np.float32).reshape(PLANES, H, W)
    wts = np.asarray(weights, dtype=np.float32).reshape(PLANES, R * R, H, W)
    lpad = np.pad(lat, ((0, 0), (PAD, PAD), (PAD, PAD)), mode="edge").astype(npdt)
    eye = np.eye(128, dtype=npdt)
    in_maps = []
    for c in range(NCORES):
        wc = wts[c * PPC : (c + 1) * PPC]  # [6, 49, 256, 256]
        # -> [row, k, plane, col] so device DMAs are contiguous per row.
        wc = np.ascontiguousarray(wc.transpose(2, 1, 0, 3).astype(npdt))
        in_maps.append(
            {
                "wt": wc,
                "lp": np.ascontiguousarray(lpad[c * PPC : (c + 1) * PPC]),
                "ident": eye,
            }
        )
    return in_maps


def _get_runner():
    """Build the Bass program and ONE sharded jit executable, cached for the
    process.  Repeated kernel() calls reuse the same loaded executable —
    creating a fresh jit per call (as run_bass_kernel_spmd does) loads a new
    executable each time and can wedge the device on the second call."""
    if "runner" in _cache:
        return _cache["runner"]

    import jax
    import concourse.mybir as mybir
    from concourse import bass2jax
    from jax.experimental.shard_map import shard_map
    from jax.sharding import Mesh, NamedSharding, PartitionSpec

    bass2jax.install_neuronx_cc_hook()
    nc = build_nc(reps=1)

    partition_name = nc.partition_id_tensor.name if nc.partition_id_tensor else None
    in_names, out_names, out_avals, zero_outs = [], [], [], []
    for alloc in nc.m.functions[0].allocations:
        if not isinstance(alloc, mybir.MemoryLocationSet):
            continue
        name = alloc.memorylocations[0].name
        if alloc.kind == "ExternalInput":
            if name != partition_name:
                in_names.append(name)
        elif alloc.kind == "ExternalOutput":
            out_names.append(name)
            shape = tuple(alloc.tensor_shape)
            dtype = mybir.dt.np(alloc.dtype)
            out_avals.append(jax.core.ShapedArray(shape, dtype))
            zero_outs.append(np.zeros(shape, dtype))
    n_params = len(in_names)
    all_in_names = list(in_names) + out_names
    if partition_name is not None:
        all_in_names.append(partition_name)

    def _body(*args):
        operands = list(args)
        if partition_name is not None:
            operands.append(bass2jax.partition_id_tensor())
        return tuple(
            bass2jax._bass_exec_p.bind(
                *operands,
                out_avals=tuple(out_avals),
                in_names=tuple(all_in_names),
                out_names=tuple(out_names),
                lowering_input_output_aliases=(),
                sim_require_finite=True,
                sim_require_nnan=True,
                nc=nc,
            )
        )

    devices = jax.devices()[:NCORES]
    mesh = Mesh(np.asarray(devices), ("core",))
    in_specs = (PartitionSpec("core"),) * (n_params + len(out_names))
    out_specs = (PartitionSpec("core"),) * len(out_names)
    sharded = jax.jit(
        shard_map(
            _body, mesh=mesh, in_specs=in_specs, out_specs=out_specs, check_rep=False
        ),
        keep_unused=True,
    )
    sh = NamedSharding(mesh, PartitionSpec("core"))
    zeros_dev = [
        jax.device_put(np.zeros((NCORES * z.shape[0], *z.shape[1:]), z.dtype), sh)
        for z in zero_outs
    ]

    def run(in_maps):
        ins_dev = [
            jax.device_put(
                np.concatenate([in_maps[c][n] for c in range(NCORES)], axis=0), sh
            )
            for n in in_names
        ]
        outs = sharded(*ins_dev, *zeros_dev)
        jax.block_until_ready(outs)
        # one output tensor: per-core [PPC, H, W] concatenated on axis 0
        return np.asarray(outs[0])

    _cache["runner"] = run
    return run


def kernel(latent, weights, window_size):
    r = int(window_size)
    assert r == R, f"kernel hardcoded for window_size={R}, got {r}"

    run = _get_runner()
    in_maps = _prep_inputs(latent, weights)
    full = run(in_maps)
    return full.reshape(B, C, H, W).astype(np.float32, copy=False)


# revision 10
# speedup vs baseline: 2.5056x; 2.5056x over previous
"""Per-pixel dynamic 7x7 filtering (BaseTextureDiffusion._diffusion_step)
on 8 Trainium2 NeuronCores.

out[b,c,h,w] = sum_k weights[b,c,k,h,w] * pad_edge(latent)[b,c,h+i,w+j],
k = i*7+j.

Sharding: the 48 (b,c) planes are independent -> 6 planes per core.
Latent is replicate-padded on host (tiny) so the device kernel does no
edge handling.  Inputs ship fp16 (halves HBM traffic; rel err ~3e-4).

Device layout per core (v2): partition dim = image rows (2 blocks of
128); free dims are (col, plane) PLANE-INNERMOST, so every column shift
j is a 12j-byte offset -> always 4B-aligned -> DVE 2x fp16 mode without
the baseline's duplicated one-element-shifted latent copies.

Engine split (vs the all-DVE baseline at ~170 us):
  - DVE computes ONLY the products: per row-shift i, ONE fused
    tensor_mul covers all 7 column taps via an overlapping access
    pattern (tap stride = col stride = 6 elems) -> 14 DVE ops/rep.
  - The 48 adds/pixel run on the otherwise-idle TensorE: products
    accumulate into PSUM via identity-stationary matmuls (f32 psum,
    which also beats the baseline's fp16 partial-sum precision).
  - ScalarE evicts PSUM -> SBUF; DMA stores f32 to HBM.
  - Weights stream as 14 x 2.75 MB DMAs on the sync-engine HWDGE ring;
    latent/output ride the scalar-engine ring so the weight stream
    never head-of-line blocks.
"""

import numpy as np

B, C, H, W = 2, 24, 256, 256
R = 7
PAD = R // 2
NCORES = 8
PLANES = B * C  # 48
PPC = PLANES // NCORES  # 6 planes per core
HP = H + 2 * PAD  # 262
WP = W + 2 * PAD  # 262
FD = PPC * W  # 1536 free elems per tap per partition
NBANK = 512  # fp32 elems per PSUM bank (matmul output limit)
DTYPE = "f16"

_cache = {}


def _split_multi_waits(nc, max_waits: int = 1):
    """walrus CoreV3 codegen in this container rejects instructions carrying
    more than one sync wait ('Too many sync wait commands').  Legalize the
    module by hoisting extra waits onto same-engine NoOps inserted directly
    before the instruction (engine stalls at the nop first — semantics
    preserved, the instruction still executes only after all conditions)."""
    import concourse.mybir as mybir

    cnt = 0
    for f in nc.m.functions:
        for b in f.blocks:
            changed = False
            new_insts = []
            for inst in b.instructions:
                si = inst.sync_info
                if si is not None and len(si.on_wait) > max_waits:
                    waits = list(si.on_wait)
                    upds = list(si.on_update)
                    chunks = [
                        waits[i : i + max_waits]
                        for i in range(0, len(waits), max_waits)
                    ]
                    for chunk in chunks[:-1]:
                        nop = mybir.InstNoOp(
                            name=f"ws_nop_{cnt}", ins=[], outs=[]
                        )
                        cnt += 1
                        nop.engine = inst.engine
                        nop.sync_info = mybir.SyncInfo(
                            on_wait=chunk, on_update=[]
                        )
                        new_insts.append(nop)
                    inst.sync_info = mybir.SyncInfo(
                        on_wait=chunks[-1], on_update=upds
                    )
                    changed = True
                new_insts.append(inst)
            if changed:
                b.instructions = new_insts
    return nc


def build_nc(
    reps: int = 1,
    loop_reps: int | None = None,
    variant: str = "full",  # "full" | "dma_only" | "w_once"
):
    """Build the per-core Bass program (SPMD; all cores run the same NEFF).

    loop_reps: if set, wrap ONE rep body in a hardware For_i loop with this
    trip count (constant NEFF size for any count; used for timing).
    """
    import concourse.bass as bass
    import concourse.mybir as mybir
    from concourse.ap import AP
    from concourse.tile import TileContext

    dt = mybir.dt.float16
    dto = mybir.dt.float32

    nc = bass.Bass("TRN2", target_bir_lowering=False, debug=False, num_devices=NCORES)
    # Host pre-transposed layouts (plane innermost):
    #   wt: [row, tap, col, plane] — one (row-block, row-shift) DMA moves 7
    #       taps = 21.5 KB per partition, fully contiguous.
    #   lp: [row, col, plane]; out: [row, col, plane].
    wt = nc.dram_tensor("wt", [H, R * R, W, PPC], dt, kind="ExternalInput").ap()
    lp = nc.dram_tensor("lp", [HP, WP, PPC], dt, kind="ExternalInput").ap()
    ident = nc.dram_tensor("ident", [128, 128], dt, kind="ExternalInput").ap()
    out = nc.dram_tensor("out", [H, W, PPC], dto, kind="ExternalOutput").ap()

    with TileContext(nc) as tc:
        with tc.tile_pool(name="pool", bufs=1) as pool, tc.psum_pool(
            name="pspool", bufs=1
        ) as pspool:
            # Identity stationary for the PE accumulation matmuls; loaded
            # once, outside the timing loop.
            id_t = pool.tile([128, 128], dt, name="id_t", tag="id", bufs=1)
            nc.sync.dma_start(out=id_t[:], in_=ident)

            def rep_body(rep):
                for blk in range(H // 128):
                    r0 = blk * 128
                    psum_t = pspool.tile(
                        [128, FD], dto, name=f"ps_{rep}_{blk}", tag="ps", bufs=2,
                    )
                    w_once_t = None
                    for i in range(R):
                        # Latent rows r0+i .. r0+i+127, plane-innermost.
                        rs = pool.tile(
                            [128, WP, PPC], dt,
                            name=f"rs_{rep}_{blk}_{i}", tag=f"rs{i}", bufs=2,
                        )
                        nc.scalar.dma_start(
                            out=rs[:], in_=lp[r0 + i : r0 + i + 128]
                        )
                        # All 7 taps of row-shift i in one 2.75 MB DMA.
                        if variant == "w_once":
                            if w_once_t is None:
                                w_once_t = pool.tile(
                                    [128, R, W, PPC], dt,
                                    name=f"wo_{rep}_{blk}", tag="wg", bufs=2,
                                )
                                nc.sync.dma_start(
                                    out=w_once_t[:],
                                    in_=wt[r0 : r0 + 128, 0:R],
                                )
                            wg = w_once_t
                        else:
                            wg = pool.tile(
                                [128, R, W, PPC], dt,
                                name=f"wg_{rep}_{blk}_{i}", tag="wg", bufs=3,
                            )
                            nc.sync.dma_start(
                                out=wg[:], in_=wt[r0 : r0 + 128, R * i : R * i + R]
                            )
                        if variant == "dma_only":
                            continue
                        # Fused product op: prod[t, c, p] = wg[t, c, p] *
                        # lp[r+i, c+t, p].  The latent operand is an
                        # overlapping AP (tap stride == col stride == PPC);
                        # runs start at 12t bytes -> 4B-aligned -> 2x mode.
                        rsa = rs[:]
                        x = AP(
                            rsa.tensor,
                            rsa.offset,
                            [list(d) for d in rsa.ap][:1]
                            + [[PPC, R], [PPC, W], [1, PPC]],
                        )
                        prod = pool.tile(
                            [128, R * FD], dt,
                            name=f"prod_{rep}_{blk}_{i}", tag="prod", bufs=2,
                        )
                        nc.vector.tensor_mul(
                            prod[:].rearrange(
                                "p (t c pl) -> p t c pl", t=R, c=W
                            ),
                            wg[:],
                            x,
                        )
                        for t in range(R):
                            for s in range(FD // NBANK):
                                nc.tensor.matmul(
                                    psum_t[:, s * NBANK : (s + 1) * NBANK],
                                    id_t[:],
                                    prod[
                                        :,
                                        t * FD + s * NBANK : t * FD + (s + 1) * NBANK,
                                    ],
                                    start=(i == 0 and t == 0),
                                    stop=(i == R - 1 and t == R - 1),
                                )
                    oacc = pool.tile(
                        [128, FD], dto, name=f"oacc_{rep}_{blk}", tag="oacc",
                        bufs=2,
                    )
                    if variant == "dma_only":
                        nc.vector.memset(oacc[:], 0.0)
                    else:
                        nc.scalar.copy(out=oacc[:], in_=psum_t[:])
                    nc.scalar.dma_start(
                        out=out[r0 : r0 + 128],
                        in_=oacc[:].rearrange("p (c pl) -> p c pl", pl=PPC),
                    )

            if loop_reps is not None:
                with tc.For_i(0, loop_reps, 1):
                    rep_body(0)
            else:
                for rep in range(reps):
                    rep_body(rep)
    _split_multi_waits(nc)
    return nc


def _prep_inputs(latent, weights, dtype: str = DTYPE):
    npdt = np.float16
    lat = np.asarray(latent, dtype=np.float32).reshape(PLANES, H, W)
    wts = np.asarray(weights, dtype=np.float32).reshape(PLANES, R * R, H, W)
    lpad = np.pad(lat, ((0, 0), (PAD, PAD), (PAD, PAD)), mode="edge").astype(npdt)
    eye = np.eye(128, dtype=npdt)
    in_maps = []
    for c in range(NCORES):
        wc = wts[c * PPC : (c + 1) * PPC]  # [6, 49, 256, 256]
        # -> [row, tap, col, plane]
        wc = np.ascontiguousarray(wc.transpose(2, 1, 3, 0).astype(npdt))
        lc = np.ascontiguousarray(
            lpad[c * PPC : (c + 1) * PPC].transpose(1, 2, 0)
        )  # [262, 262, 6]
        in_maps.append({"wt": wc, "lp": lc, "ident": eye})
    return in_maps


def _get_runner():
    """Build the Bass program and ONE sharded jit executable, cached for the
    process.  Repeated kernel() calls reuse the same loaded executable —
    creating a fresh jit per call (as run_bass_kernel_spmd does) loads a new
    executable each time and can wedge the device on the second call."""
    if "runner" in _cache:
        return _cache["runner"]

    import jax
    import concourse.mybir as mybir
    from concourse import bass2jax
    from jax.experimental.shard_map import shard_map
    from jax.sharding import Mesh, NamedSharding, PartitionSpec

    bass2jax.install_neuronx_cc_hook()
    nc = build_nc(reps=1)

    partition_name = nc.partition_id_tensor.name if nc.partition_id_tensor else None
    in_names, out_names, out_avals, zero_outs = [], [], [], []
    for alloc in nc.m.functions[0].allocations:
        if not isinstance(alloc, mybir.MemoryLocationSet):
            continue
        name = alloc.memorylocations[0].name
        if alloc.kind == "ExternalInput":
            if name != partition_name:
                in_names.append(name)
        elif alloc.kind == "ExternalOutput":
            out_names.append(name)
            shape = tuple(alloc.tensor_shape)
            dtype = mybir.dt.np(alloc.dtype)
            out_avals.append(jax.core.ShapedArray(shape, dtype))
            zero_outs.append(np.zeros(shape, dtype))
    n_params = len(in_names)
    all_in_names = list(in_names) + out_names
    if partition_name is not None:
        all_in_names.append(partition_name)

    def _body(*args):
        operands = list(args)
        if partition_name is not None:
            operands.append(bass2jax.partition_id_tensor())
        return tuple(
            bass2jax._bass_exec_p.bind(
                *operands,
                out_avals=tuple(out_avals),
                in_names=tuple(all_in_names),
                out_names=tuple(out_names),
                lowering_input_output_aliases=(),
                sim_require_finite=True,
                sim_require_nnan=True,
                nc=nc,
            )
        )

    devices = jax.devices()[:NCORES]
    mesh = Mesh(np.asarray(devices), ("core",))
    in_specs = (PartitionSpec("core"),) * (n_params + len(out_names))
    out_specs = (PartitionSpec("core"),) * len(out_names)
    sharded = jax.jit(
        shard_map(
            _body, mesh=mesh, in_specs=in_specs, out_specs=out_specs, check_rep=False
        ),
        keep_unused=True,
    )
    sh = NamedSharding(mesh, PartitionSpec("core"))
    zeros_dev = [
        jax.device_put(np.zeros((NCORES * z.shape[0], *z.shape[1:]), z.dtype), sh)
        for z in zero_outs
    ]

    def run(in_maps):
        ins_dev = [
            jax.device_put(
                np.concatenate([in_maps[c][n] for c in range(NCORES)], axis=0), sh
            )
            for n in in_names
        ]
        outs = sharded(*ins_dev, *zeros_dev)
        jax.block_until_ready(outs)
        # one output tensor: per-core [H, W, PPC] concatenated on axis 0
        return np.asarray(outs[0])

    _cache["runner"] = run
    return run


def kernel(latent, weights, window_size):
    r = int(window_size)
    assert r == R, f"kernel hardcoded for window_size={R}, got {r}"

    run = _get_runner()
    in_maps = _prep_inputs(latent, weights)
    full = run(in_maps)  # [NCORES*H, W, PPC]
    full = full.reshape(NCORES, H, W, PPC)
    full = full.transpose(0, 3, 1, 2)  # [NCORES, PPC, H, W]
    return (
        full.reshape(B, C, H, W).astype(np.float32, copy=False)
    )


# revision 14
# speedup vs baseline: 2.7825x; 1.1105x over previous
"""Per-pixel dynamic 7x7 filtering (BaseTextureDiffusion._diffusion_step)
on 8 Trainium2 NeuronCores.

out[b,c,h,w] = sum_k weights[b,c,k,h,w] * pad_edge(latent)[b,c,h+i,w+j],
k = i*7+j.

Sharding: the 48 (b,c) planes are independent -> 6 planes per core.
Latent is replicate-padded on host (tiny) so the device kernel does no
edge handling.  Inputs ship fp16 (halves HBM traffic; rel err ~3e-4).

Device layout per core (v2): partition dim = image rows (2 blocks of
128); free dims are (col, plane) PLANE-INNERMOST, so every column shift
j is a 12j-byte offset -> always 4B-aligned -> DVE 2x fp16 mode without
the baseline's duplicated one-element-shifted latent copies.

Engine split (vs the all-DVE baseline at ~170 us):
  - DVE computes ONLY the products: per row-shift i, ONE fused
    tensor_mul covers all 7 column taps via an overlapping access
    pattern (tap stride = col stride = 6 elems) -> 14 DVE ops/rep.
  - The 48 adds/pixel run on the otherwise-idle TensorE: products
    accumulate into PSUM via identity-stationary matmuls (f32 psum,
    which also beats the baseline's fp16 partial-sum precision).
  - ScalarE evicts PSUM -> SBUF; DMA stores f32 to HBM.
  - Weights stream as 14 x 2.75 MB DMAs on the sync-engine HWDGE ring;
    latent/output ride the scalar-engine ring so the weight stream
    never head-of-line blocks.
"""

import numpy as np

B, C, H, W = 2, 24, 256, 256
R = 7
PAD = R // 2
NCORES = 8
PLANES = B * C  # 48
PPC = PLANES // NCORES  # 6 planes per core
HP = H + 2 * PAD  # 262
WP = W + 2 * PAD  # 262
FD = PPC * W  # 1536 free elems per tap per partition
NBANK = 512  # fp32 elems per PSUM bank (matmul output limit)
DTYPE = "f16"

_cache = {}


def _split_multi_waits(nc, max_waits: int = 1):
    """walrus CoreV3 codegen in this container rejects instructions carrying
    more than one sync wait ('Too many sync wait commands').  Legalize the
    module by hoisting extra waits onto same-engine NoOps inserted directly
    before the instruction (engine stalls at the nop first — semantics
    preserved, the instruction still executes only after all conditions)."""
    import concourse.mybir as mybir

    cnt = 0
    for f in nc.m.functions:
        for b in f.blocks:
            changed = False
            new_insts = []
            for inst in b.instructions:
                si = inst.sync_info
                if si is not None and len(si.on_wait) > max_waits:
                    waits = list(si.on_wait)
                    upds = list(si.on_update)
                    chunks = [
                        waits[i : i + max_waits]
                        for i in range(0, len(waits), max_waits)
                    ]
                    for chunk in chunks[:-1]:
                        nop = mybir.InstNoOp(
                            name=f"ws_nop_{cnt}", ins=[], outs=[]
                        )
                        cnt += 1
                        nop.engine = inst.engine
                        nop.sync_info = mybir.SyncInfo(
                            on_wait=chunk, on_update=[]
                        )
                        new_insts.append(nop)
                    inst.sync_info = mybir.SyncInfo(
                        on_wait=chunks[-1], on_update=upds
                    )
                    changed = True
                new_insts.append(inst)
            if changed:
                b.instructions = new_insts
    return nc


def build_nc(
    reps: int = 1,
    loop_reps: int | None = None,
    variant: str = "full",  # "full" | "dma_only" | "w_once"
):
    """Build the per-core Bass program (SPMD; all cores run the same NEFF).

    loop_reps: if set, wrap ONE rep body in a hardware For_i loop with this
    trip count (constant NEFF size for any count; used for timing).
    """
    import concourse.bass as bass
    import concourse.mybir as mybir
    from concourse.ap import AP
    from concourse.tile import TileContext

    dt = mybir.dt.float16
    dto = mybir.dt.float32

    nc = bass.Bass("TRN2", target_bir_lowering=False, debug=False, num_devices=NCORES)
    # Host pre-transposed layouts (plane innermost):
    #   wt: [row, tap, col, plane] — one (row-block, row-shift) DMA moves 7
    #       taps = 21.5 KB per partition, fully contiguous.
    #   lp: [row, col, plane]; out: [row, col, plane].
    wt = nc.dram_tensor("wt", [H, R * R, W, PPC], dt, kind="ExternalInput").ap()
    lp = nc.dram_tensor("lp", [HP, WP, PPC], dt, kind="ExternalInput").ap()
    ident = nc.dram_tensor("ident", [128, 128], dt, kind="ExternalInput").ap()
    out = nc.dram_tensor("out", [H, W, PPC], dt, kind="ExternalOutput").ap()

    with TileContext(nc) as tc:
        with tc.tile_pool(name="pool", bufs=1) as pool, tc.psum_pool(
            name="pspool", bufs=1
        ) as pspool:
            # Identity stationary for the PE accumulation matmuls; loaded
            # once, outside the timing loop.
            id_t = pool.tile([128, 128], dt, name="id_t", tag="id", bufs=1)
            nc.sync.dma_start(out=id_t[:], in_=ident)

            def rep_body(rep):
                for blk in range(H // 128):
                    r0 = blk * 128
                    # Row-shifted latent tiles, one HBM load per shift.
                    rs_tiles = {}
                    for i in range(R):
                        t = pool.tile(
                            [128, WP, PPC], dt,
                            name=f"rs_{rep}_{blk}_{i}", tag=f"rs{i}", bufs=2,
                        )
                        nc.scalar.dma_start(
                            out=t[:], in_=lp[r0 + i : r0 + i + 128]
                        )
                        rs_tiles[i] = t
                    psum_t = pspool.tile(
                        [128, FD], dto, name=f"ps_{rep}_{blk}", tag="ps", bufs=2,
                    )
                    w_once_t = None
                    for i in range(R):
                        rs = rs_tiles[i]
                        # 7 taps of row-shift i: per-tap slice DMAs (finer
                        # completion granularity pipelines better than one
                        # 2.75 MB transfer).
                        if variant == "w_once":
                            if w_once_t is None:
                                w_once_t = pool.tile(
                                    [128, R, W, PPC], dt,
                                    name=f"wo_{rep}_{blk}", tag="wg", bufs=2,
                                )
                                nc.sync.dma_start(
                                    out=w_once_t[:],
                                    in_=wt[r0 : r0 + 128, 0:R],
                                )
                            wg = w_once_t
                        else:
                            wg = pool.tile(
                                [128, R, W, PPC], dt,
                                name=f"wg_{rep}_{blk}_{i}", tag="wg", bufs=4,
                            )
                            for t in range(R):
                                nc.sync.dma_start(
                                    out=wg[:, t],
                                    in_=wt[r0 : r0 + 128, R * i + t],
                                )
                        if variant == "dma_only":
                            continue
                        # Fused product op: prod[t, c, p] = wg[t, c, p] *
                        # lp[r+i, c+t, p].  The latent operand is an
                        # overlapping AP (tap stride == col stride == PPC);
                        # runs start at 12t bytes -> 4B-aligned -> 2x mode.
                        rsa = rs[:]
                        x = AP(
                            rsa.tensor,
                            rsa.offset,
                            [list(d) for d in rsa.ap][:1]
                            + [[PPC, R], [PPC, W], [1, PPC]],
                        )
                        prod = pool.tile(
                            [128, R * FD], dt,
                            name=f"prod_{rep}_{blk}_{i}", tag="prod", bufs=2,
                        )
                        nc.vector.tensor_mul(
                            prod[:].rearrange(
                                "p (t c pl) -> p t c pl", t=R, c=W
                            ),
                            wg[:],
                            x,
                        )
                        for t in range(R):
                            for s in range(FD // NBANK):
                                nc.tensor.matmul(
                                    psum_t[:, s * NBANK : (s + 1) * NBANK],
                                    id_t[:],
                                    prod[
                                        :,
                                        t * FD + s * NBANK : t * FD + (s + 1) * NBANK,
                                    ],
                                    start=(i == 0 and t == 0),
                                    stop=(i == R - 1 and t == R - 1),
                                )
                    # fp16 output (host upcasts): halves store traffic; adds
                    # ~4e-4 quantization, still far below the 2e-2 gate.
                    oacc = pool.tile(
                        [128, FD], dt, name=f"oacc_{rep}_{blk}", tag="oacc",
                        bufs=2,
                    )
                    if variant == "dma_only":
                        nc.vector.memset(oacc[:], 0.0)
                    else:
                        nc.scalar.copy(out=oacc[:], in_=psum_t[:])
                    nc.scalar.dma_start(
                        out=out[r0 : r0 + 128],
                        in_=oacc[:].rearrange("p (c pl) -> p c pl", pl=PPC),
                    )

            if loop_reps is not None:
                with tc.For_i(0, loop_reps, 1):
                    rep_body(0)
            else:
                for rep in range(reps):
                    rep_body(rep)
    _split_multi_waits(nc)
    return nc


def _prep_inputs(latent, weights, dtype: str = DTYPE):
    npdt = np.float16
    lat = np.asarray(latent, dtype=np.float32).reshape(PLANES, H, W)
    wts = np.asarray(weights, dtype=np.float32).reshape(PLANES, R * R, H, W)
    lpad = np.pad(lat, ((0, 0), (PAD, PAD), (PAD, PAD)), mode="edge").astype(npdt)
    eye = np.eye(128, dtype=npdt)
    in_maps = []
    for c in range(NCORES):
        wc = wts[c * PPC : (c + 1) * PPC]  # [6, 49, 256, 256]
        # -> [row, tap, col, plane]
        wc = np.ascontiguousarray(wc.transpose(2, 1, 3, 0).astype(npdt))
        lc = np.ascontiguousarray(
            lpad[c * PPC : (c + 1) * PPC].transpose(1, 2, 0)
        )  # [262, 262, 6]
        in_maps.append({"wt": wc, "lp": lc, "ident": eye})
    return in_maps


def _get_runner():
    """Build the Bass program and ONE sharded jit executable, cached for the
    process.  Repeated kernel() calls reuse the same loaded executable —
    creating a fresh jit per call (as run_bass_kernel_spmd does) loads a new
    executable each time and can wedge the device on the second call."""
    if "runner" in _cache:
        return _cache["runner"]

    import jax
    import concourse.mybir as mybir
    from concourse import bass2jax
    from jax.experimental.shard_map import shard_map
    from jax.sharding import Mesh, NamedSharding, PartitionSpec

    bass2jax.install_neuronx_cc_hook()
    nc = build_nc(reps=1)

    partition_name = nc.partition_id_tensor.name if nc.partition_id_tensor else None
    in_names, out_names, out_avals, zero_outs = [], [], [], []
    for alloc in nc.m.functions[0].allocations:
        if not isinstance(alloc, mybir.MemoryLocationSet):
            continue
        name = alloc.memorylocations[0].name
        if alloc.kind == "ExternalInput":
            if name != partition_name:
                in_names.append(name)
        elif alloc.kind == "ExternalOutput":
            out_names.append(name)
            shape = tuple(alloc.tensor_shape)
            dtype = mybir.dt.np(alloc.dtype)
            out_avals.append(jax.core.ShapedArray(shape, dtype))
            zero_outs.append(np.zeros(shape, dtype))
    n_params = len(in_names)
    all_in_names = list(in_names) + out_names
    if partition_name is not None:
        all_in_names.append(partition_name)

    def _body(*args):
        operands = list(args)
        if partition_name is not None:
            operands.append(bass2jax.partition_id_tensor())
        return tuple(
            bass2jax._bass_exec_p.bind(
                *operands,
                out_avals=tuple(out_avals),
                in_names=tuple(all_in_names),
                out_names=tuple(out_names),
                lowering_input_output_aliases=(),
                sim_require_finite=True,
                sim_require_nnan=True,
                nc=nc,
            )
        )

    devices = jax.devices()[:NCORES]
    mesh = Mesh(np.asarray(devices), ("core",))
    in_specs = (PartitionSpec("core"),) * (n_params + len(out_names))
    out_specs = (PartitionSpec("core"),) * len(out_names)
    sharded = jax.jit(
        shard_map(
            _body, mesh=mesh, in_specs=in_specs, out_specs=out_specs, check_rep=False
        ),
        keep_unused=True,
    )
    sh = NamedSharding(mesh, PartitionSpec("core"))
    zeros_dev = [
        jax.device_put(np.zeros((NCORES * z.shape[0], *z.shape[1:]), z.dtype), sh)
        for z in zero_outs
    ]

    def run(in_maps):
        ins_dev = [
            jax.device_put(
                np.concatenate([in_maps[c][n] for c in range(NCORES)], axis=0), sh
            )
            for n in in_names
        ]
        outs = sharded(*ins_dev, *zeros_dev)
        jax.block_until_ready(outs)
        # one output tensor: per-core [H, W, PPC] concatenated on axis 0
        return np.asarray(outs[0])

    _cache["runner"] = run
    return run


def kernel(latent, weights, window_size):
    r = int(window_size)
    assert r == R, f"kernel hardcoded for window_size={R}, got {r}"

    run = _get_runner()
    in_maps = _prep_inputs(latent, weights)
    full = run(in_maps)  # [NCORES*H, W, PPC]
    full = full.reshape(NCORES, H, W, PPC)
    full = full.transpose(0, 3, 1, 2)  # [NCORES, PPC, H, W]
    return (
        full.reshape(B, C, H, W).astype(np.float32, copy=False)
    )
